# revision 25
# baseline (speedup 1.0000x reference)
"""Multi-head self-attention (B=4, S=2048, D=1024, H=16, causal) on 8 TRN2
NeuronCores.

Sharding: batch x head-group. Core c handles batch b = c//2 and head-group
g = c%2 (8 heads = 512 of the 1024 q/k/v dims). Each core computes a partial
output [S, D] (its head-group's contribution through w_o); the host sums the
two partials per batch and adds b_o.

Per-core kernel (all matmuls bf16, fp32 accumulate), software-pipelined so
the PE never idles long enough for the HAM clock gate to re-throttle: the
projection matmuls of q-block qb+1 and (deferred) output projections are
emitted interleaved with the (ACT-bound) attention of q-block qb. Output
projections are deferred two q-blocks (qb2 runs outproj 0; qb3 runs outproj
1 and 2) because late q-blocks have the largest exp load and need the most
PE filler.

Attention per q-block (512 queries):
  S^T[k,q] = Kt.T @ Qt with two heads packed into the PE array via
  tile_position row groups; causally-dead query columns of diagonal k-tiles
  are clipped out of the score matmul, the exp, the mask multiply, and the
  PV matmul (only the 128-wide diagonal block needs the triangular mask);
  merged exp over two k-tiles on ACT with fused 1/sqrt(dk) scale (no max
  subtraction: |scores| <~ 6 so exp is safe); O'^T += V'.T @ P^T where V'
  carries a ones column so the softmax denominator accumulates for free;
  normalize reads the PSUM accumulators directly (reciprocal + gpsimd
  partition broadcast) and writes bf16 ots for the output projection.
"""

import numpy as np

import concourse.bass as bass
import concourse.mybir as mybir
from concourse import bacc
from concourse.tile import TileContext
from concourse.bass_utils import run_bass_kernel_spmd

B, S, D, H = 4, 2048, 1024, 16
DK = D // H          # 64
N_CORES = 8
GD = D // 2          # 512 dims per head-group
SCALE = 1.0 / float(np.sqrt(DK))

F32 = mybir.dt.float32
BF16 = mybir.dt.bfloat16
EXP = mybir.ActivationFunctionType.Exp

_cache = {}


def _build():
    if "nc" in _cache:
        return _cache["nc"]

    nc = bacc.Bacc("TRN2", target_bir_lowering=False, debug=False,
                   num_devices=N_CORES)

    xT = nc.dram_tensor("xT", (D, S), BF16, kind="ExternalInput")
    wq_t = nc.dram_tensor("wq_t", (D, GD), BF16, kind="ExternalInput")
    wk_t = nc.dram_tensor("wk_t", (D, GD), BF16, kind="ExternalInput")
    wv_t = nc.dram_tensor("wv_t", (D, GD), BF16, kind="ExternalInput")
    wo_t = nc.dram_tensor("wo_t", (GD, D), BF16, kind="ExternalInput")
    masks = nc.dram_tensor("masks", (128, 128), BF16, kind="ExternalInput")
    out_p = nc.dram_tensor("out_p", (S, D), BF16, kind="ExternalOutput")

    xT_r = xT.rearrange("(t p) s -> p t s", p=128)        # [128, 8, 2048]
    wq_r = wq_t.rearrange("(t p) d -> p t d", p=128)      # [128, 8, 512]
    wk_r = wk_t.rearrange("(t p) d -> p t d", p=128)
    wv_r = wv_t.rearrange("(t p) d -> p t d", p=128)
    wo_r = wo_t.rearrange("(t p) d -> p t d", p=128)      # [128, 4, 1024]

    with TileContext(nc) as tc:
        with (
            tc.tile_pool(name="pers", bufs=1) as pers,
            tc.tile_pool(name="wp", bufs=1) as wp,
            tc.tile_pool(name="xq", bufs=2) as xq,
            tc.tile_pool(name="wkp", bufs=2) as wkp,
            tc.tile_pool(name="ps", bufs=2, space="PSUM") as ps,
        ):
            # persistent K^T (d-major) and V' (s-major, 65 cols/head)
            kt = [pers.tile([128, S], BF16, name=f"kt{t}") for t in range(4)]
            vp = [pers.tile([128, 8 * (DK + 1)], BF16, name=f"vp{i}")
                  for i in range(16)]

            wq_sb = wp.tile([128, 8, GD], BF16)
            wk_sb = wp.tile([128, 8, GD], BF16)
            wv_sb = wp.tile([128, 8, GD], BF16)
            wo_sb = wp.tile([128, 4, D], BF16)
            mask_sb = wp.tile([128, 1, 128], BF16)
            ones_c = wp.tile([128, 1], F32)
            # input DMAs on the sync queue (keeps the ACT queue exp-only);
            # first-needed first, wo deferred (not used until ~qb2)
            for t in range(4):
                csl = slice(t * 128, (t + 1) * 128)
                nc.sync.dma_start(out=wq_sb[:, :, csl], in_=wq_r[:, :, csl])
                nc.sync.dma_start(out=wk_sb[:, :, csl], in_=wk_r[:, :, csl])
            nc.vector.memset(ones_c, 1.0)

            def emit_wv_load():
                # scalar queue (parallel with sync wq/wk), emitted after the
                # first chain so xh/wq win the HBM bandwidth race at t=0
                nc.scalar.dma_start(out=wv_sb, in_=wv_r)
                nc.scalar.dma_start(out=mask_sb[:, 0, :], in_=masks[:, :])
            wo_loaded = [False]

            def emit_wo_load():
                if not wo_loaded[0]:
                    nc.sync.dma_start(out=wo_sb, in_=wo_r)
                    wo_loaded[0] = True

            xh_by_qb = {}
            qts_by_qb = {}
            ots_by_qb = {}

            def emit_xh(qb):
                """Issue x DMAs for q-block qb (sliced so the first chain can
                start after the first 256KB lands)."""
                qs = slice(qb * 512, (qb + 1) * 512)
                xh = []
                for h in range(2):
                    xt = xq.tile([128, 4, 512], BF16, tag="xh", bufs=4,
                                 name=f"xh{qb}_{h}")
                    for i in range(4):
                        nc.gpsimd.dma_start(out=xt[:, i, :],
                                            in_=xT_r[:, 4 * h + i, qs])
                    xh.append(xt)
                xh_by_qb[qb] = xh
                qts_by_qb[qb] = []

            def emit_qk_chain(qb, t):
                qs = slice(qb * 512, (qb + 1) * 512)
                xh = xh_by_qb[qb]
                qt_t = xq.tile([128, 512], BF16, tag="qts", bufs=8,
                               name=f"qts{qb}_{t}")
                for dst, wsb in ((qt_t, wq_sb), (None, wk_sb)):
                    pst = ps.tile([128, 512], F32, tag="mm512", bufs=2,
                                  name=f"pp{qb}_{t}")
                    for e in range(8):
                        nc.tensor.matmul(
                            pst,
                            wsb[:, e, t * 128:(t + 1) * 128],
                            xh[e // 4][:, e % 4, :],
                            start=(e == 0), stop=(e == 7),
                        )
                    if dst is None:
                        nc.vector.tensor_copy(kt[t][:, qs], pst)
                    else:
                        nc.vector.tensor_copy(dst, pst)
                qts_by_qb[qb].append(qt_t)

            def emit_v_chain(qb, part):
                xh = xh_by_qb[qb]
                sidx = 4 * qb + part
                psv = ps.tile([128, 512], F32, tag="mm512", bufs=2, name=f"pv{sidx}")
                for e in range(8):
                    nc.tensor.matmul(
                        psv,
                        xh[e // 4][:, e % 4, part * 128:(part + 1) * 128],
                        wv_sb[:, e, :],
                        start=(e == 0), stop=(e == 7),
                    )
                vt = vp[sidx].rearrange("p (h c) -> p h c", c=DK + 1)
                nc.vector.tensor_copy(
                    vt[:, :, 0:DK], psv.rearrange("p (h d) -> p h d", d=DK)
                )
                nc.vector.tensor_copy(
                    vt[:, :, DK], ones_c.broadcast_to([128, 8])
                )

            st_tiles = {}
            ot_tiles = {}

            def clip(qb, ki):
                # causally-dead query columns of diagonal k-tiles
                return 128 * (ki - 4 * qb) if ki >= 4 * qb else 0

            def emit_st(it):
                qb, pair, m = it
                qts = qts_by_qb[qb]
                sts = []
                for j in (0, 1):
                    ki = 2 * m + j
                    off = clip(qb, ki)
                    ksl = slice(ki * 128, (ki + 1) * 128)
                    # heads A and B side by side in one 2-bank psum tensor:
                    # the two row-group matmuls share a slot, stay adjacent
                    # in the schedule, and co-execute on disjoint PE
                    # sub-arrays
                    st = ps.tile([128, 1024], F32, tag="st",
                                 name=f"st{qb}_{pair}_{m}_{j}")
                    nc.tensor.matmul(
                        st[:, off:512],
                        kt[pair][0:DK, ksl], qts[pair][0:DK, off:512],
                        start=True, stop=True, tile_position=(0, 0),
                    )
                    nc.tensor.matmul(
                        st[:, 512 + off:1024],
                        kt[pair][DK:128, ksl], qts[pair][DK:128, off:512],
                        start=True, stop=True, tile_position=(64, 0),
                    )
                    sts.append(st)
                st_tiles[it] = sts

            def emit_rest(it):
                qb, pair, m = it
                n_merge = 2 * qb + 2
                hA, hB = 2 * pair, 2 * pair + 1
                if m == 0:
                    ot_tiles[(qb, pair)] = (
                        ps.tile([DK + 1, 512], F32, tag="ot2", bufs=2,
                                name=f"otA{qb}_{pair}"),
                        ps.tile([DK + 1, 512], F32, tag="ot2", bufs=2,
                                name=f"otB{qb}_{pair}"),
                    )
                otA, otB = ot_tiles[(qb, pair)]
                sts = st_tiles.pop(it)
                for j in (0, 1):
                    ki = 2 * m + j
                    off = clip(qb, ki)
                    st = sts[j]
                    pt = wkp.tile([128, 1024], BF16, tag="pt",
                                  bufs=4, name=f"pt{qb}_{pair}_{m}_{j}")
                    pt_h = pt.rearrange("p (h c) -> p h c", h=2)
                    st_h = st.rearrange("p (h c) -> p h c", h=2)
                    nc.scalar.activation(pt_h[:, :, off:512],
                                         st_h[:, :, off:512], EXP, scale=SCALE)
                    if ki >= 4 * qb:
                        # triangular mask on the 128-wide diagonal block only
                        nc.vector.tensor_mul(
                            pt_h[:, :, off:off + 128],
                            pt_h[:, :, off:off + 128],
                            mask_sb.broadcast_to([128, 2, 128]),
                        )
                    first = (m == 0 and j == 0)
                    last = (m == n_merge - 1 and j == 1)
                    nc.tensor.matmul(
                        otA[:, off:512], vp[ki][:, hA * 65:hA * 65 + 65],
                        pt[:, off:512],
                        start=first, stop=last,
                    )
                    nc.tensor.matmul(
                        otB[:, off:512], vp[ki][:, hB * 65:hB * 65 + 65],
                        pt[:, 512 + off:1024],
                        start=first, stop=last,
                    )

            def emit_norm(qb, pair):
                # reciprocal-normalize, half-pipelined across DVE and GpSimd
                # (recip of half B overlaps the partition broadcast of half A)
                ots = ots_by_qb[qb]
                otA, otB = ot_tiles.pop((qb, pair))
                rc = wkp.tile([1, 1024], F32, tag="rc", bufs=2,
                              name=f"rc{qb}_{pair}")
                nc.vector.tensor_copy(rc[:, 0:512], otA[DK:DK + 1, :])
                nc.vector.tensor_copy(rc[:, 512:1024], otB[DK:DK + 1, :])
                rb = wkp.tile([64, 1024], F32, tag="rb", bufs=2,
                              name=f"rb{qb}_{pair}")
                for hl, ot in ((0, otA), (1, otB)):
                    hs = slice(hl * 512, (hl + 1) * 512)
                    nc.vector.reciprocal_approx_fast(rb[0:1, hs], rc[:, hs])
                    nc.gpsimd.partition_broadcast(rb[:, hs], rb[0:1, hs])
                for hl, ot in ((0, otA), (1, otB)):
                    nc.vector.tensor_mul(
                        ots[pair][hl * DK:(hl + 1) * DK, :],
                        ot[0:DK, :], rb[:, hl * 512:(hl + 1) * 512],
                    )

            ostg_by = {}

            def emit_outproj_half(qb, j, half):
                ots = ots_by_qb[qb]
                if half == 0:
                    ostg_by[(qb, j)] = wkp.tile([128, 1024], BF16, tag="ostg",
                                                bufs=2, name=f"ostg{qb}_{j}")
                ostg = ostg_by[(qb, j)]
                psc = ps.tile([128, 512], F32, tag="mm512", bufs=2,
                              name=f"po{half}_{qb}_{j}")
                for di in range(4):
                    lhs = ots[di][:, j * 128:(j + 1) * 128]
                    nc.tensor.matmul(
                        psc, lhs, wo_sb[:, di, half * 512:(half + 1) * 512],
                        start=(di == 0), stop=(di == 3))
                nc.vector.tensor_copy(
                    ostg[:, half * 512:(half + 1) * 512], psc)
                if half == 1:
                    sidx = 4 * qb + j
                    ostg_by.pop((qb, j))
                    nc.sync.dma_start(
                        out=out_p[sidx * 128:(sidx + 1) * 128, :], in_=ostg
                    )

            # ---- software-pipelined emission with 1-iteration S^T lookahead.
            # Filler (next q-block's projections + deferred output
            # projections) is distributed BETWEEN items, emitted ahead of the
            # stall-prone PV matmuls in the strict-FIFO PE queue, so the PE
            # has runnable work whenever a PV waits on the exp chain.
            items = []
            for qb in range(4):
                for pair in range(4):
                    for m in range(2 * qb + 2):
                        items.append((qb, pair, m))

            def pair_filler(qb, pair):
                fill = []
                if qb < 3:
                    fill.append(lambda q=qb + 1, t=pair: emit_qk_chain(q, t))
                    fill.append(lambda q=qb + 1, t=pair: emit_v_chain(q, t))
                # ALL deferred outproj runs inside qb3 (its exp load leaves
                # the PE ~20us short of work); (2,2)/(2,3) are kept for the
                # tail to hide the last norm chain
                if qb == 3:
                    for oq in (0, 1):
                        fill += [lambda h=h, p=pair, q=oq:
                                 emit_outproj_half(q, p, h) for h in range(2)]
                    if pair < 2:
                        fill += [lambda h=h, p=pair: emit_outproj_half(2, p, h)
                                 for h in range(2)]
                return fill

            emit_xh(0)
            emit_qk_chain(0, 0)
            emit_wv_load()
            for t in range(1, 4):
                emit_qk_chain(0, t)
            emit_xh(1)
            for t in range(4):
                emit_v_chain(0, t)
            emit_st(items[0])
            fill = []
            n_fill_done = 0
            for idx, it in enumerate(items):
                qb, pair, m = it
                n_items = 2 * qb + 2
                if m == 0:
                    if pair == 0:
                        ots_by_qb[qb] = [
                            xq.tile([128, 512], BF16, tag="ots", bufs=16,
                                    name=f"ots{qb}_{t}") for t in range(4)
                        ]
                        if qb == 1:
                            emit_wo_load()
                    if pair == 2 and qb <= 1:
                        # x DMAs for qb+2 issued a full q-block before its
                        # projection chains run
                        emit_xh(qb + 2)
                    fill = pair_filler(qb, pair)
                    n_fill_done = 0
                if idx + 1 < len(items):
                    emit_st(items[idx + 1])
                while n_fill_done * n_items < len(fill) * (m + 1):
                    fill[n_fill_done]()
                    n_fill_done += 1
                emit_rest(it)
                if m == n_items - 1:  # last merge of this pair
                    emit_norm(qb, pair)
            for j in (2, 3):
                for h in range(2):
                    emit_outproj_half(2, j, h)
            for j in range(4):
                for h in range(2):
                    emit_outproj_half(3, j, h)

    nc.compile()
    _cache["nc"] = nc
    return nc


def _build_masks():
    # tri[kr, qc] = 1 iff qc >= kr (triangular block for diagonal tiles)
    import ml_dtypes
    kr = np.arange(128)[:, None]
    qc = np.arange(128)[None, :]
    return (qc >= kr).astype(ml_dtypes.bfloat16)


def _in_maps(x, w_q, w_k, w_v, w_o, masks):
    import ml_dtypes
    bf = ml_dtypes.bfloat16
    maps = []
    for core in range(N_CORES):
        b, g = core // 2, core % 2
        sl = slice(g * GD, (g + 1) * GD)
        maps.append({
            "xT": np.ascontiguousarray(x[b].T).astype(bf),
            "wq_t": np.ascontiguousarray(w_q[sl, :].T).astype(bf),
            "wk_t": np.ascontiguousarray(w_k[sl, :].T).astype(bf),
            "wv_t": np.ascontiguousarray(w_v[sl, :].T).astype(bf),
            "wo_t": np.ascontiguousarray(w_o[:, sl].T).astype(bf),
            "masks": masks,
        })
    return maps


def kernel(x, w_q, w_k, w_v, w_o, b_o):
    x = np.asarray(x, dtype=np.float32)
    w_q = np.asarray(w_q, dtype=np.float32)
    w_k = np.asarray(w_k, dtype=np.float32)
    w_v = np.asarray(w_v, dtype=np.float32)
    w_o = np.asarray(w_o, dtype=np.float32)
    b_o = np.asarray(b_o, dtype=np.float32)

    nc = _build()
    in_maps = _in_maps(x, w_q, w_k, w_v, w_o, _build_masks())

    res = run_bass_kernel_spmd(nc, in_maps, core_ids=list(range(N_CORES)),
                               trace=False)

    out = np.empty((B, S, D), dtype=np.float32)
    for b in range(B):
        out[b] = (res.results[2 * b]["out_p"].astype(np.float32)
                  + res.results[2 * b + 1]["out_p"].astype(np.float32))
    out += b_o[None, None, :]
    return out


# revision 30
# speedup vs baseline: 1.1669x; 1.1669x over previous
"""Multi-head self-attention (B=4, S=2048, D=1024, H=16, causal) on 8 TRN2
NeuronCores.

Sharding: batch x head-group. Core c handles batch b = c//2 and head-group
g = c%2 (8 heads = 512 of the 1024 q/k/v dims). Each core computes a partial
output [S, D] (its head-group's contribution through w_o); the host sums the
two partials per batch and adds b_o.

Per-core kernel (all matmuls bf16, fp32 accumulate), software-pipelined so
the PE never idles long enough for the HAM clock gate to re-throttle: the
projection matmuls of q-block qb+1 and (deferred) output projections are
emitted interleaved with the (ACT-bound) attention of q-block qb. Output
projections are deferred two q-blocks (qb2 runs outproj 0; qb3 runs outproj
1 and 2) because late q-blocks have the largest exp load and need the most
PE filler.

Attention per q-block (512 queries):
  S^T[k,q] = Kt.T @ Qt with two heads packed into the PE array via
  tile_position row groups; causally-dead query columns of diagonal k-tiles
  are clipped out of the score matmul, the exp, the mask multiply, and the
  PV matmul (only the 128-wide diagonal block needs the triangular mask);
  merged exp over two k-tiles on ACT with fused 1/sqrt(dk) scale (no max
  subtraction: |scores| <~ 6 so exp is safe); O'^T += V'.T @ P^T where V'
  carries a ones column so the softmax denominator accumulates for free;
  normalize reads the PSUM accumulators directly (reciprocal + gpsimd
  partition broadcast) and writes bf16 ots for the output projection.
"""

import numpy as np

import concourse.bass as bass
import concourse.mybir as mybir
from concourse import bacc
from concourse.tile import TileContext
from concourse.bass_utils import run_bass_kernel_spmd

B, S, D, H = 4, 2048, 1024, 16
DK = D // H          # 64
N_CORES = 8
GD = D // 2          # 512 dims per head-group
SCALE = 1.0 / float(np.sqrt(DK))

F32 = mybir.dt.float32
BF16 = mybir.dt.bfloat16
EXP = mybir.ActivationFunctionType.Exp

_cache = {}


def _build():
    if "nc" in _cache:
        return _cache["nc"]

    nc = bacc.Bacc("TRN2", target_bir_lowering=False, debug=False,
                   num_devices=N_CORES)

    xT = nc.dram_tensor("xT", (D, S), BF16, kind="ExternalInput")
    wq_t = nc.dram_tensor("wq_t", (D, GD), BF16, kind="ExternalInput")
    wk_t = nc.dram_tensor("wk_t", (D, GD), BF16, kind="ExternalInput")
    wv_t = nc.dram_tensor("wv_t", (D, GD), BF16, kind="ExternalInput")
    wo_t = nc.dram_tensor("wo_t", (GD, D), BF16, kind="ExternalInput")
    masks = nc.dram_tensor("masks", (128, 128), BF16, kind="ExternalInput")
    out_p = nc.dram_tensor("out_p", (S, D), BF16, kind="ExternalOutput")

    xT_r = xT.rearrange("(t p) s -> p t s", p=128)        # [128, 8, 2048]
    wq_r = wq_t.rearrange("(t p) d -> p t d", p=128)      # [128, 8, 512]
    wk_r = wk_t.rearrange("(t p) d -> p t d", p=128)
    wv_r = wv_t.rearrange("(t p) d -> p t d", p=128)
    wo_r = wo_t.rearrange("(t p) d -> p t d", p=128)      # [128, 4, 1024]

    with TileContext(nc) as tc:
        with (
            tc.tile_pool(name="pers", bufs=1) as pers,
            tc.tile_pool(name="wp", bufs=1) as wp,
            tc.tile_pool(name="xq", bufs=2) as xq,
            tc.tile_pool(name="wkp", bufs=2) as wkp,
            tc.tile_pool(name="ps", bufs=2, space="PSUM") as ps,
        ):
            # persistent K^T (d-major) and V' (s-major, 65 cols/head)
            kt = [pers.tile([128, S], BF16, name=f"kt{t}") for t in range(4)]
            vp = [pers.tile([128, 8 * (DK + 1)], BF16, name=f"vp{i}")
                  for i in range(16)]

            wq_sb = wp.tile([128, 8, GD], BF16)
            wk_sb = wp.tile([128, 8, GD], BF16)
            wv_sb = wp.tile([128, 8, GD], BF16)
            wo_sb = wp.tile([128, 4, D], BF16)
            mask_sb = wp.tile([128, 1, 128], BF16)
            ones_c = wp.tile([128, 1], F32)
            # input DMAs on the sync queue (keeps the ACT queue exp-only);
            # first-needed first, wo deferred (not used until ~qb2)
            for t in range(4):
                csl = slice(t * 128, (t + 1) * 128)
                nc.sync.dma_start(out=wq_sb[:, :, csl], in_=wq_r[:, :, csl])
                nc.sync.dma_start(out=wk_sb[:, :, csl], in_=wk_r[:, :, csl])
            nc.vector.memset(ones_c, 1.0)

            def emit_wv_load():
                # scalar queue (parallel with sync wq/wk), emitted after the
                # first chain so xh/wq win the HBM bandwidth race at t=0
                nc.scalar.dma_start(out=wv_sb, in_=wv_r)
                nc.scalar.dma_start(out=mask_sb[:, 0, :], in_=masks[:, :])
            wo_loaded = [False]

            def emit_wo_load():
                if not wo_loaded[0]:
                    nc.sync.dma_start(out=wo_sb, in_=wo_r)
                    wo_loaded[0] = True

            xh_by_qb = {}
            qts_by_qb = {}
            ots_by_qb = {}

            def emit_xh(qb):
                """Issue x DMAs for q-block qb (sliced so the first chain can
                start after the first 256KB lands)."""
                qs = slice(qb * 512, (qb + 1) * 512)
                xh = []
                for h in range(2):
                    xt = xq.tile([128, 4, 512], BF16, tag="xh", bufs=4,
                                 name=f"xh{qb}_{h}")
                    for i in range(4):
                        nc.gpsimd.dma_start(out=xt[:, i, :],
                                            in_=xT_r[:, 4 * h + i, qs])
                    xh.append(xt)
                xh_by_qb[qb] = xh
                qts_by_qb[qb] = []

            def emit_qk_chain(qb, t):
                qs = slice(qb * 512, (qb + 1) * 512)
                xh = xh_by_qb[qb]
                qt_t = xq.tile([128, 512], BF16, tag="qts", bufs=8,
                               name=f"qts{qb}_{t}")
                for dst, wsb in ((qt_t, wq_sb), (None, wk_sb)):
                    pst = ps.tile([128, 512], F32, tag="mm512", bufs=2,
                                  name=f"pp{qb}_{t}")
                    for e in range(8):
                        nc.tensor.matmul(
                            pst,
                            wsb[:, e, t * 128:(t + 1) * 128],
                            xh[e // 4][:, e % 4, :],
                            start=(e == 0), stop=(e == 7),
                        )
                    if dst is None:
                        nc.vector.tensor_copy(kt[t][:, qs], pst)
                    else:
                        nc.vector.tensor_copy(dst, pst)
                qts_by_qb[qb].append(qt_t)

            def emit_v_chain(qb, part):
                xh = xh_by_qb[qb]
                sidx = 4 * qb + part
                psv = ps.tile([128, 512], F32, tag="mm512", bufs=2, name=f"pv{sidx}")
                for e in range(8):
                    nc.tensor.matmul(
                        psv,
                        xh[e // 4][:, e % 4, part * 128:(part + 1) * 128],
                        wv_sb[:, e, :],
                        start=(e == 0), stop=(e == 7),
                    )
                vt = vp[sidx].rearrange("p (h c) -> p h c", c=DK + 1)
                nc.vector.tensor_copy(
                    vt[:, :, 0:DK], psv.rearrange("p (h d) -> p h d", d=DK)
                )
                nc.vector.tensor_copy(
                    vt[:, :, DK], ones_c.broadcast_to([128, 8])
                )

            st_tiles = {}
            ot_tiles = {}

            def clip(qb, ki):
                # causally-dead query columns of diagonal k-tiles
                return 128 * (ki - 4 * qb) if ki >= 4 * qb else 0

            def emit_st(it):
                qb, pair, m = it
                qts = qts_by_qb[qb]
                sts = []
                for j in (0, 1):
                    ki = 2 * m + j
                    off = clip(qb, ki)
                    ksl = slice(ki * 128, (ki + 1) * 128)
                    # heads A and B side by side in one 2-bank psum tensor:
                    # the two row-group matmuls share a slot, stay adjacent
                    # in the schedule, and co-execute on disjoint PE
                    # sub-arrays
                    st = ps.tile([128, 1024], F32, tag="st",
                                 name=f"st{qb}_{pair}_{m}_{j}")
                    nc.tensor.matmul(
                        st[:, off:512],
                        kt[pair][0:DK, ksl], qts[pair][0:DK, off:512],
                        start=True, stop=True, tile_position=(0, 0),
                    )
                    nc.tensor.matmul(
                        st[:, 512 + off:1024],
                        kt[pair][DK:128, ksl], qts[pair][DK:128, off:512],
                        start=True, stop=True, tile_position=(64, 0),
                    )
                    sts.append(st)
                st_tiles[it] = sts

            def emit_rest(it):
                qb, pair, m = it
                n_merge = 2 * qb + 2
                hA, hB = 2 * pair, 2 * pair + 1
                if m == 0:
                    ot_tiles[(qb, pair)] = (
                        ps.tile([DK + 1, 512], F32, tag="ot2", bufs=2,
                                name=f"otA{qb}_{pair}"),
                        ps.tile([DK + 1, 512], F32, tag="ot2", bufs=2,
                                name=f"otB{qb}_{pair}"),
                    )
                otA, otB = ot_tiles[(qb, pair)]
                sts = st_tiles.pop(it)
                for j in (0, 1):
                    ki = 2 * m + j
                    off = clip(qb, ki)
                    st = sts[j]
                    pt = wkp.tile([128, 1024], BF16, tag="pt",
                                  bufs=4, name=f"pt{qb}_{pair}_{m}_{j}")
                    pt_h = pt.rearrange("p (h c) -> p h c", h=2)
                    st_h = st.rearrange("p (h c) -> p h c", h=2)
                    nc.scalar.activation(pt_h[:, :, off:512],
                                         st_h[:, :, off:512], EXP, scale=SCALE)
                    if ki >= 4 * qb:
                        # triangular mask on the 128-wide diagonal block only
                        nc.vector.tensor_mul(
                            pt_h[:, :, off:off + 128],
                            pt_h[:, :, off:off + 128],
                            mask_sb.broadcast_to([128, 2, 128]),
                        )
                    first = (m == 0 and j == 0)
                    last = (m == n_merge - 1 and j == 1)
                    nc.tensor.matmul(
                        otA[:, off:512], vp[ki][:, hA * 65:hA * 65 + 65],
                        pt[:, off:512],
                        start=first, stop=last,
                    )
                    nc.tensor.matmul(
                        otB[:, off:512], vp[ki][:, hB * 65:hB * 65 + 65],
                        pt[:, 512 + off:1024],
                        start=first, stop=last,
                    )

            def emit_norm(qb, pair):
                # reciprocal-normalize, half-pipelined across DVE and GpSimd
                # (recip of half B overlaps the partition broadcast of half A)
                ots = ots_by_qb[qb]
                otA, otB = ot_tiles.pop((qb, pair))
                rc = wkp.tile([1, 1024], F32, tag="rc", bufs=2,
                              name=f"rc{qb}_{pair}")
                nc.vector.tensor_copy(rc[:, 0:512], otA[DK:DK + 1, :])
                nc.vector.tensor_copy(rc[:, 512:1024], otB[DK:DK + 1, :])
                rb = wkp.tile([64, 1024], F32, tag="rb", bufs=2,
                              name=f"rb{qb}_{pair}")
                for hl, ot in ((0, otA), (1, otB)):
                    hs = slice(hl * 512, (hl + 1) * 512)
                    nc.vector.reciprocal_approx_fast(rb[0:1, hs], rc[:, hs])
                    nc.gpsimd.partition_broadcast(rb[:, hs], rb[0:1, hs])
                for hl, ot in ((0, otA), (1, otB)):
                    nc.vector.tensor_mul(
                        ots[pair][hl * DK:(hl + 1) * DK, :],
                        ot[0:DK, :], rb[:, hl * 512:(hl + 1) * 512],
                    )

            ostg_by = {}

            def emit_outproj_half(qb, j, half):
                ots = ots_by_qb[qb]
                if half == 0:
                    ostg_by[(qb, j)] = wkp.tile([128, 1024], BF16, tag="ostg",
                                                bufs=2, name=f"ostg{qb}_{j}")
                ostg = ostg_by[(qb, j)]
                psc = ps.tile([128, 512], F32, tag="mm512", bufs=2,
                              name=f"po{half}_{qb}_{j}")
                for di in range(4):
                    lhs = ots[di][:, j * 128:(j + 1) * 128]
                    nc.tensor.matmul(
                        psc, lhs, wo_sb[:, di, half * 512:(half + 1) * 512],
                        start=(di == 0), stop=(di == 3))
                nc.vector.tensor_copy(
                    ostg[:, half * 512:(half + 1) * 512], psc)
                if half == 1:
                    sidx = 4 * qb + j
                    ostg_by.pop((qb, j))
                    nc.sync.dma_start(
                        out=out_p[sidx * 128:(sidx + 1) * 128, :], in_=ostg
                    )

            # ---- software-pipelined emission with 1-iteration S^T lookahead.
            # Filler (next q-block's projections + deferred output
            # projections) is distributed BETWEEN items, emitted ahead of the
            # stall-prone PV matmuls in the strict-FIFO PE queue, so the PE
            # has runnable work whenever a PV waits on the exp chain.
            items = []
            for qb in range(4):
                for pair in range(4):
                    for m in range(2 * qb + 2):
                        items.append((qb, pair, m))

            def pair_filler(qb, pair):
                fill = []
                if qb < 3:
                    fill.append(lambda q=qb + 1, t=pair: emit_qk_chain(q, t))
                    fill.append(lambda q=qb + 1, t=pair: emit_v_chain(q, t))
                # deferred outproj: qb2 runs stiles of qb0; qb3 runs stiles
                # of qb1 and qb2 (except (2,3), kept for the tail to hide the
                # last norm chain)
                if qb == 2:
                    fill += [lambda h=h, p=pair: emit_outproj_half(0, p, h)
                             for h in range(2)]
                if qb == 3:
                    fill += [lambda h=h, p=pair: emit_outproj_half(1, p, h)
                             for h in range(2)]
                    if pair < 2:
                        fill += [lambda h=h, p=pair: emit_outproj_half(2, p, h)
                                 for h in range(2)]
                return fill

            emit_xh(0)
            emit_qk_chain(0, 0)
            emit_wv_load()
            for t in range(1, 4):
                emit_qk_chain(0, t)
            emit_xh(1)
            for t in range(4):
                emit_v_chain(0, t)
            emit_st(items[0])
            fill = []
            n_fill_done = 0
            for idx, it in enumerate(items):
                qb, pair, m = it
                n_items = 2 * qb + 2
                if m == 0:
                    if pair == 0:
                        ots_by_qb[qb] = [
                            xq.tile([128, 512], BF16, tag="ots", bufs=12,
                                    name=f"ots{qb}_{t}") for t in range(4)
                        ]
                        if qb == 1:
                            emit_wo_load()
                    if pair == 2 and qb <= 1:
                        # x DMAs for qb+2 issued a full q-block before its
                        # projection chains run
                        emit_xh(qb + 2)
                    fill = pair_filler(qb, pair)
                    n_fill_done = 0
                if idx + 1 < len(items):
                    emit_st(items[idx + 1])
                while n_fill_done * n_items < len(fill) * (m + 1):
                    fill[n_fill_done]()
                    n_fill_done += 1
                emit_rest(it)
                if m == n_items - 1:  # last merge of this pair
                    emit_norm(qb, pair)
            for j in (2, 3):
                for h in range(2):
                    emit_outproj_half(2, j, h)
            for j in range(4):
                for h in range(2):
                    emit_outproj_half(3, j, h)

    nc.compile()
    _cache["nc"] = nc
    return nc


def _build_masks():
    # tri[kr, qc] = 1 iff qc >= kr (triangular block for diagonal tiles)
    import ml_dtypes
    kr = np.arange(128)[:, None]
    qc = np.arange(128)[None, :]
    return (qc >= kr).astype(ml_dtypes.bfloat16)


def _in_maps(x, w_q, w_k, w_v, w_o, masks):
    import ml_dtypes
    bf = ml_dtypes.bfloat16
    maps = []
    for core in range(N_CORES):
        b, g = core // 2, core % 2
        sl = slice(g * GD, (g + 1) * GD)
        maps.append({
            "xT": np.ascontiguousarray(x[b].T).astype(bf),
            "wq_t": np.ascontiguousarray(w_q[sl, :].T).astype(bf),
            "wk_t": np.ascontiguousarray(w_k[sl, :].T).astype(bf),
            "wv_t": np.ascontiguousarray(w_v[sl, :].T).astype(bf),
            "wo_t": np.ascontiguousarray(w_o[:, sl].T).astype(bf),
            "masks": masks,
        })
    return maps


def kernel(x, w_q, w_k, w_v, w_o, b_o):
    x = np.asarray(x, dtype=np.float32)
    w_q = np.asarray(w_q, dtype=np.float32)
    w_k = np.asarray(w_k, dtype=np.float32)
    w_v = np.asarray(w_v, dtype=np.float32)
    w_o = np.asarray(w_o, dtype=np.float32)
    b_o = np.asarray(b_o, dtype=np.float32)

    nc = _build()
    in_maps = _in_maps(x, w_q, w_k, w_v, w_o, _build_masks())

    res = run_bass_kernel_spmd(nc, in_maps, core_ids=list(range(N_CORES)),
                               trace=False)

    out = np.empty((B, S, D), dtype=np.float32)
    for b in range(B):
        out[b] = (res.results[2 * b]["out_p"].astype(np.float32)
                  + res.results[2 * b + 1]["out_p"].astype(np.float32))
    out += b_o[None, None, :]
    return out
